# revision 1
# baseline (speedup 1.0000x reference)
"""Trainium2 Bass kernel for nn_Backward_12094627905824 (MLP trunk + gumbel-argmax
mixture sampling). Data-parallel over 8 NeuronCores: batch B=262144 is sharded
32768 rows/core; the small MLP / head weights are replicated.

Math per batch row b (reference semantics):
  h = relu chain: 3 -> 128 -> 256 -> 200
  mu/sig/pai[g,d] = heads (25 comps x 4 dims), pai/sigma through abs
  idx[d] = argmax_g log(pai+1e-12) + gumbel[b,g,d]
  out[b,d] = rand[b,d]*|sig[idx,d]| + mu[idx,d]

On-device reformulation (argmax-invariant): score = |pai_raw| * exp(gumbel);
selection via one-hot (score >= rowmax) mask-and-sum.
"""
import numpy as np

import concourse.bass as bass
import concourse.mybir as mybir
import bass_rust
from concourse.tile import TileContext
from concourse.bass_utils import run_bass_kernel_spmd

NCORES = 8
B, G, D = 262144, 25, 4
GD = G * D                       # 100
H1, H2, H3 = 128, 256, 200
BS = B // NCORES                 # 32768 rows per core
NB = 512                         # batch columns per compute tile
NT = BS // NB                    # 64 tiles
NSUB = NB // 128                 # 4 sub-blocks of 128 rows

F32 = mybir.dt.float32
F32R = mybir.dt.float32r


def _split_multi_waits(nc):
    # walrus CoreV3 codegen accepts only one sync-wait per instruction; Tile's
    # exit drain waits once per active proc. Split into single-wait drains.
    for bb in nc.main_func.blocks:
        insts = list(bb.instructions)
        out = []
        changed = False
        for inst in insts:
            si = inst.sync_info
            if si is not None and len(si.on_wait) > 1:
                waits = list(si.on_wait)
                for k, w in enumerate(waits[:-1]):
                    d = mybir.InstDrain(name=f"{inst.name}-sw{k}", ins=[], outs=[])
                    d.engine = inst.engine
                    d.sync_info = bass_rust.SyncInfo(on_wait=[w], on_update=[])
                    nc.register_instruction(d)
                    out.append(d)
                si.on_wait = [waits[-1]]
                changed = True
            out.append(inst)
        if changed:
            bb.instructions = out


def _build_nc():
    nc = bass.Bass(trn_type="TRN2")

    x0t = nc.dram_tensor("x0t", [3, BS], F32R, kind="ExternalInput")
    gum = nc.dram_tensor("gum", [BS, GD], F32, kind="ExternalInput")
    rnd = nc.dram_tensor("rnd", [BS, D], F32, kind="ExternalInput")
    w1t = nc.dram_tensor("w1t", [3, H1], F32R, kind="ExternalInput")
    b1 = nc.dram_tensor("b1", [H1, 1], F32, kind="ExternalInput")
    w2t = nc.dram_tensor("w2t", [H1, H2], F32R, kind="ExternalInput")
    b2 = nc.dram_tensor("b2", [H2, 1], F32, kind="ExternalInput")
    w3t = nc.dram_tensor("w3t", [H2, H3], F32R, kind="ExternalInput")   # rows = h2 feat
    b3 = nc.dram_tensor("b3", [H3, 1], F32, kind="ExternalInput")
    wh = nc.dram_tensor("wh", [H3, 300], F32R, kind="ExternalInput")  # [mu|sig|pai]
    onesr = nc.dram_tensor("onesr", [2, NB], F32R, kind="ExternalInput")
    whbias = nc.dram_tensor("whbias", [2, 300], F32R, kind="ExternalInput")
    out_d = nc.dram_tensor("out", [BS, D], F32, kind="ExternalOutput")

    from contextlib import ExitStack
    with TileContext(nc) as tc, ExitStack() as ctx:
        const = ctx.enter_context(tc.tile_pool(name="const", bufs=1))
        io = ctx.enter_context(tc.tile_pool(name="io", bufs=3))
        act = ctx.enter_context(tc.tile_pool(name="act", bufs=4))
        samp = ctx.enter_context(tc.tile_pool(name="samp", bufs=3))
        ptrunk = ctx.enter_context(tc.tile_pool(name="ptrunk", bufs=3, space="PSUM"))
        pheads = ctx.enter_context(tc.tile_pool(name="pheads", bufs=4, space="PSUM"))

        # --- load weights once ---
        w1t_s = const.tile([3, H1], F32R)
        nc.sync.dma_start(out=w1t_s, in_=w1t[:, :])
        b1_s = const.tile([H1, 1], F32)
        nc.sync.dma_start(out=b1_s, in_=b1[:, :])
        w2t_s = const.tile([H1, H2], F32R)
        nc.sync.dma_start(out=w2t_s, in_=w2t[:, :])
        b2a_s = const.tile([128, 1], F32, tag="b2a")
        nc.sync.dma_start(out=b2a_s, in_=b2[0:128, :])
        b2b_s = const.tile([128, 1], F32, tag="b2b")
        nc.sync.dma_start(out=b2b_s, in_=b2[128:256, :])
        w3ta_s = const.tile([128, H3], F32R, tag="w3ta")   # h2 feats 0:128
        nc.sync.dma_start(out=w3ta_s, in_=w3t[0:128, :])
        w3tb_s = const.tile([128, H3], F32R, tag="w3tb")   # h2 feats 128:256
        nc.sync.dma_start(out=w3tb_s, in_=w3t[128:256, :])
        b3a_s = const.tile([128, 1], F32, tag="b3a")
        nc.sync.dma_start(out=b3a_s, in_=b3[0:128, :])
        b3b_s = const.tile([72, 1], F32, tag="b3b")
        nc.sync.dma_start(out=b3b_s, in_=b3[128:200, :])
        wha_s = const.tile([128, 300], F32R, tag="wha")    # h3 feats 0:128
        nc.sync.dma_start(out=wha_s, in_=wh[0:128, :])
        whb_s = const.tile([72, 300], F32R, tag="whb")     # h3 feats 128:200
        nc.sync.dma_start(out=whb_s, in_=wh[128:200, :])
        bias_row_s = const.tile([2, 300], F32R, tag="biasrow")
        nc.sync.dma_start(out=bias_row_s, in_=whbias[:, :])
        ones_s = const.tile([2, NB], F32R, tag="ones")
        nc.sync.dma_start(out=ones_s, in_=onesr[:, :])



        for it in range(NT):
            b0 = it * NB

            # --- input DMAs ---
            x_s = io.tile([3, NB], F32R, tag="x")
            nc.sync.dma_start(out=x_s, in_=x0t[:, b0:b0 + NB])
            gum_s = io.tile([128, NSUB, GD], F32, tag="gum")
            nc.sync.dma_start(
                out=gum_s,
                in_=gum[b0:b0 + NB, :].rearrange("(s p) e -> p s e", s=NSUB),
            )
            rnd_s = io.tile([128, NSUB, D], F32, tag="rnd")
            nc.sync.dma_start(
                out=rnd_s,
                in_=rnd[b0:b0 + NB, :].rearrange("(s p) d -> p s d", s=NSUB),
            )

            # --- trunk ---
            h1p = ptrunk.tile([128, NB], F32, tag="pt")
            nc.tensor.matmul(h1p, lhsT=w1t_s[:, :], rhs=x_s[:, :],
                             start=True, stop=True)
            h1 = act.tile([128, NB], F32R, tag="h1")
            nc.scalar.activation(h1, h1p, func=mybir.ActivationFunctionType.Relu,
                                 bias=b1_s[:, :], scale=1.0)

            h2ap = ptrunk.tile([128, NB], F32, tag="pt")
            nc.tensor.matmul(h2ap, lhsT=w2t_s[:, 0:128], rhs=h1[:, :],
                             start=True, stop=True)
            h2a = act.tile([128, NB], F32R, tag="h2a")
            nc.scalar.activation(h2a, h2ap, func=mybir.ActivationFunctionType.Relu,
                                 bias=b2a_s[:, :], scale=1.0)

            h2bp = ptrunk.tile([128, NB], F32, tag="pt")
            nc.tensor.matmul(h2bp, lhsT=w2t_s[:, 128:256], rhs=h1[:, :],
                             start=True, stop=True)
            h2b = act.tile([128, NB], F32R, tag="h2b")
            nc.scalar.activation(h2b, h2bp, func=mybir.ActivationFunctionType.Relu,
                                 bias=b2b_s[:, :], scale=1.0)

            h3ap = ptrunk.tile([128, NB], F32, tag="pt")
            nc.tensor.matmul(h3ap, lhsT=w3ta_s[:, 0:128], rhs=h2a[:, :],
                             start=True, stop=False)
            nc.tensor.matmul(h3ap, lhsT=w3tb_s[:, 0:128], rhs=h2b[:, :],
                             start=False, stop=True)
            h3a = act.tile([128, NB], F32R, tag="h3a")
            nc.scalar.activation(h3a, h3ap, func=mybir.ActivationFunctionType.Relu,
                                 bias=b3a_s[:, :], scale=1.0)

            h3bp = ptrunk.tile([72, NB], F32, tag="pt")
            nc.tensor.matmul(h3bp, lhsT=w3ta_s[:, 128:200], rhs=h2a[:, :],
                             start=True, stop=False)
            nc.tensor.matmul(h3bp, lhsT=w3tb_s[:, 128:200], rhs=h2b[:, :],
                             start=False, stop=True)
            h3b = act.tile([72, NB], F32R, tag="h3b")
            nc.scalar.activation(h3b, h3bp,
                                 func=mybir.ActivationFunctionType.Relu,
                                 bias=b3b_s[:, :], scale=1.0)

            # --- heads: psum[s] = [mu(100) | sig(100) | pai(100)] per 128-row sub
            hp = []
            for s in range(NSUB):
                hps = pheads.tile([128, 300], F32, tag="hp")
                c0, c1 = s * 128, (s + 1) * 128
                nc.tensor.matmul(hps, lhsT=h3a[:, c0:c1], rhs=wha_s[:, :],
                                 start=True, stop=False)
                nc.tensor.matmul(hps, lhsT=h3b[:, c0:c1], rhs=whb_s[:, :],
                                 start=False, stop=False)
                nc.tensor.matmul(hps, lhsT=ones_s[:, c0:c1], rhs=bias_row_s[:, :],
                                 start=False, stop=True)
                hp.append(hps)

            # --- sampling ---
            ex = samp.tile([128, NSUB, GD], F32, tag="ex")
            nc.scalar.activation(ex, gum_s, func=mybir.ActivationFunctionType.Exp)

            absp = samp.tile([128, NSUB, GD], F32, tag="absp")
            for s in range(NSUB):
                nc.scalar.activation(absp[:, s], hp[s][:, 200:300],
                                     func=mybir.ActivationFunctionType.Abs)
            # score = |pai_raw| * exp(gumbel)
            sc = samp.tile([128, NSUB, GD], F32, tag="sc")
            nc.vector.tensor_mul(sc, absp, ex)

            # rowmax over g per (sub, d):  view (p, s, d, g)
            sc_v = sc.rearrange("p s (g d) -> p s d g", g=G)
            smax = samp.tile([128, NSUB, D], F32, tag="smax")
            nc.vector.tensor_reduce(smax, sc_v, axis=mybir.AxisListType.X,
                                    op=mybir.AluOpType.max)

            # one-hot: oh = (score >= smax)
            oh = samp.tile([128, NSUB, GD], F32, tag="oh")
            smax_b = smax.unsqueeze(3).broadcast_to([128, NSUB, D, G])
            nc.vector.tensor_tensor(
                out=oh.rearrange("p s (g d) -> p s d g", g=G),
                in0=sc_v,
                in1=smax_b, op=mybir.AluOpType.is_ge)

            # masked select-sum of mu and sig: pms = [mu|sig] * oh
            pms = samp.tile([128, NSUB, 2, GD], F32, tag="pms")
            for s in range(NSUB):
                oh_b = oh[:, s].unsqueeze(1).broadcast_to([128, 2, GD])
                nc.vector.tensor_mul(pms[:, s], hp[s][:, 0:200]
                                     .rearrange("p (h e) -> p h e", h=2), oh_b)

            sel = samp.tile([128, NSUB, 2, D], F32, tag="sel")
            nc.vector.tensor_reduce(
                sel, pms.rearrange("p s h (g d) -> p s h d g", g=G),
                axis=mybir.AxisListType.X, op=mybir.AluOpType.add)

            # out = rnd * |sig_sel| + mu_sel
            siga = samp.tile([128, NSUB, D], F32, tag="siga")
            nc.vector.scalar_tensor_tensor(
                out=siga, in0=sel[:, :, 1, :], scalar=-1.0, in1=sel[:, :, 1, :],
                op0=mybir.AluOpType.mult, op1=mybir.AluOpType.max)
            ot = samp.tile([128, NSUB, D], F32, tag="ot")
            nc.vector.tensor_mul(ot, rnd_s, siga)
            nc.vector.tensor_add(ot, ot, sel[:, :, 0, :])

            nc.sync.dma_start(
                out=out_d[b0:b0 + NB, :].rearrange("(s p) d -> p s d", s=NSUB),
                in_=ot)

    _split_multi_waits(nc)
    return nc


_NC_CACHE = None
LAST_RESULT = None


def kernel(x0, rand, gumbel, W1, b1, W2, b2, W3, b3,
           Wmu, bmu, Wsig, bsig, Wpai, bpai):
    global _NC_CACHE, LAST_RESULT
    if _NC_CACHE is None:
        _NC_CACHE = _build_nc()
    nc = _NC_CACHE

    x0 = np.ascontiguousarray(np.asarray(x0, np.float32))
    rand = np.ascontiguousarray(np.asarray(rand, np.float32))
    gumbel = np.ascontiguousarray(np.asarray(gumbel, np.float32))

    # stacked head weights [201, 300]: rows 0..199 = h3 feats, row 200 = bias;
    # col = head*100 + g*4 + d
    WH = np.zeros((H3 + 1, 300), np.float32)
    for hd, (W, b) in enumerate([(Wmu, bmu), (Wsig, bsig), (Wpai, bpai)]):
        WH[:H3, hd * GD:(hd + 1) * GD] = np.asarray(W, np.float32).reshape(GD, H3).T
        WH[H3, hd * GD:(hd + 1) * GD] = np.asarray(b, np.float32).reshape(GD)

    def _split10(a):
        """hi = a with mantissa truncated to 10 explicit bits (exactly
        representable in fp32r), lo = exact residual."""
        a = np.ascontiguousarray(a, np.float32)
        hi = (a.view(np.uint32) & np.uint32(0xFFFFE000)).view(np.float32)
        return hi, np.ascontiguousarray(a - hi)

    w2hi = np.ascontiguousarray(np.asarray(W2, np.float32).T)
    w3hi = np.ascontiguousarray(np.asarray(W3, np.float32).T)
    whhi = np.ascontiguousarray(WH[:H3])
    bhi, blo = _split10(WH[H3:H3 + 1])
    wmats = {
        "w1t": np.ascontiguousarray(np.asarray(W1, np.float32).T),
        "b1": np.asarray(b1, np.float32).reshape(H1, 1),
        "w2t": w2hi,
        "b2": np.asarray(b2, np.float32).reshape(H2, 1),
        "w3t": w3hi,
        "b3": np.asarray(b3, np.float32).reshape(H3, 1),
        "wh": whhi,
        "whbias": np.ascontiguousarray(np.vstack([bhi, blo])),
    }

    in_maps = []
    for c in range(NCORES):
        sl = slice(c * BS, (c + 1) * BS)
        m = {
            "onesr": np.ones((2, NB), np.float32),
            "x0t": np.ascontiguousarray(x0[sl].T),
            "gum": np.ascontiguousarray(gumbel[sl].reshape(BS, GD)),
            "rnd": np.ascontiguousarray(rand[sl]),
        }
        m.update(wmats)
        in_maps.append(m)

    res = run_bass_kernel_spmd(nc, in_maps, core_ids=list(range(NCORES)))
    LAST_RESULT = res
    out = np.concatenate([res.results[c]["out"] for c in range(NCORES)], axis=0)
    return out.astype(np.float32)

